# revision 1
# baseline (speedup 1.0000x reference)
"""DoRA linear layer on 8 TRN2 NeuronCores.

out = (magnitude / ||W + s*B@A||_row) * (x @ (W + s*B@A)^T),  s = alpha/rank = 2.

Identity used: the reference's
    dora_out + base_out = mag_norm_scale * (base_out + s * lora_out)
                        = scale_o * (x @ W_adapted^T)
so the kernel runs ONE big fp32r matmul x @ W_ad^T (with the rank-16 term
added as an extra PSUM-accumulated matmul) and a per-out-column scale.

Sharding: data-parallel on tokens (8192 tokens -> 1024/core); W/A/B/mag
replicated. Host side only reshapes/transposes (layout prep) and rounds
fp32 -> fp32r bit format (the dtype the tensor engine consumes).

Row norms of W_ad are computed on-device from the expansion
  ||W + B2@A||^2_row = rowsum(W*W) + 2*rowsum((W@A^T) * B2) + rowsum((B2@G) * B2)
with B2 = s*B, G = A@A^T.  rowsum(W*W) and W@A^T come from one fused fp16
matmul per W^T tile (gram diag + cross term), everything else is tiny.
"""

import sys

sys.path.insert(0, "/opt/trn_rl_repo")

import numpy as np

import concourse.bass as bass  # noqa: F401  (import keeps bass registered)
from concourse import bacc
import concourse.mybir as mybir
from concourse.tile import TileContext
from concourse.bass_utils import run_bass_kernel_spmd
from concourse.masks import make_identity

FP32 = mybir.dt.float32
F32R = mybir.dt.float32r
FP16 = mybir.dt.float16

NCORES = 8
TOK = 8192          # 4 * 2048 tokens
TPC = TOK // NCORES  # 1024 tokens per core
DIN = 4096
DOUT = 4096
RANK = 16
SCALING = 32.0 / 16

NI = DIN // 128     # 32 contraction blocks
NCOL = 8            # output columns of 512
OC = DOUT // NCOL   # 512
NT = TPC // 128     # 8 token tiles per core
H = 8               # ib-chunk size (W-tile working set)
NH = NI // H        # 4 chunks per column


def _round_f32r(x: np.ndarray) -> np.ndarray:
    """Round-to-nearest-even fp32 -> fp32r bit format (11 explicit mantissa
    bits, low 12 bits zero) — matches the PE's own input rounding."""
    u = np.ascontiguousarray(x, dtype=np.float32).view(np.uint32)
    odd = (u >> np.uint32(12)) & np.uint32(1)
    r = (u + np.uint32(0x7FF) + odd) & np.uint32(0xFFFFF000)
    return r.view(np.float32)


def _build_program(ncol_limit=NCOL, skip_prologue=False):
    nc = bacc.Bacc("TRN2", target_bir_lowering=False, debug=False,
                   num_devices=NCORES)

    xt_d = nc.dram_tensor("xt", [128, NI, TPC], FP32, kind="ExternalInput")
    wt_d = nc.dram_tensor("wt", [NCOL, NI, 128, OC], FP32, kind="ExternalInput")
    wh_d = nc.dram_tensor("wh", [128, 32, DIN], FP16, kind="ExternalInput")
    at_d = nc.dram_tensor("at", [128, NI, RANK], FP32, kind="ExternalInput")
    b2t_d = nc.dram_tensor("b2t", [RANK, DOUT], FP32, kind="ExternalInput")
    mag_d = nc.dram_tensor("mag", [1, DOUT], FP32, kind="ExternalInput")
    out_d = nc.dram_tensor("out", [TPC, DOUT], FP32, kind="ExternalOutput")
    srow_d = nc.dram_tensor("srow_scratch", [NCOL, OC], FP32)
    n1_d = nc.dram_tensor("n1_scratch", [NCOL, OC], FP32)

    with TileContext(nc) as tc:
        with (
            tc.tile_pool(name="const", bufs=1) as const,
            tc.tile_pool(name="xtp", bufs=1) as xtp,
            tc.tile_pool(name="wp", bufs=10) as wp,
            tc.tile_pool(name="outp", bufs=10) as outp,
            tc.tile_pool(name="whp", bufs=3) as whp,
            tc.tile_pool(name="b2tp", bufs=2) as b2tp,
            tc.tile_pool(name="sbcp", bufs=2) as sbcp,
            tc.tile_pool(name="mp", bufs=3, space="PSUM") as mp,
            tc.tile_pool(name="np", bufs=2, space="PSUM") as npp,
        ):
            ident = const.tile([128, 128], FP32)
            make_identity(nc, ident)

            aT = const.tile([128, NI, RANK], F32R)
            nc.sync.dma_start(aT[:], at_d[:].bitcast(F32R))
            ones16 = const.tile([RANK, 1], FP32)
            nc.vector.memset(ones16[:], 1.0)

            # resident x^T  [i_part, i_blk, tok] — four tiles so consumers
            # of early i-blocks need not wait for the whole 16 MiB load
            xTq = []
            for q in range(4):
                xq = xtp.tile([128, 8, TPC], F32R, name=f"xTq{q}")
                nc.sync.dma_start(xq[:], xt_d[:, q * 8:(q + 1) * 8, :].bitcast(F32R))
                xTq.append(xq)

            def xT(ib):
                return xTq[ib // 8][:, ib % 8, :]

            # xa^T = (x @ A^T)^T  [rank, tok]
            xaT = const.tile([RANK, TPC], F32R)
            for q in range(2):
                ps_xa = mp.tile([RANK, 512], FP32, tag="mp", name=f"psxa{q}")
                for ib in range(NI):
                    nc.tensor.matmul(
                        ps_xa[:], aT[:, ib, :], xT(ib)[:, q * 512:(q + 1) * 512],
                        start=(ib == 0), stop=(ib == NI - 1))
                nc.vector.tensor_copy(xaT[:, q * 512:(q + 1) * 512], ps_xa[:])

            # G = A @ A^T  [rank, rank]
            ps_g = mp.tile([RANK, RANK], FP32, tag="mp", name="psg")
            for ib in range(NI):
                nc.tensor.matmul(ps_g[:], aT[:, ib, :], aT[:, ib, :],
                                 start=(ib == 0), stop=(ib == NI - 1))
            g_sb = const.tile([RANK, RANK], F32R)
            nc.vector.tensor_copy(g_sb[:], ps_g[:])

            # n1 = rowsum(W*W) per out row, via ACT Square+accumulate over a
            # fp16 copy of W in natural layout; 4 chunk-partials per subtile
            n1p = const.tile([128, 4], FP32)
            n1col = const.tile([128, 4], FP32)
            n1row = const.tile([4, 128], FP32)
            # row-space [1, OC] norm pieces
            prod2 = const.tile([RANK, OC], FP32)
            prod3 = const.tile([RANK, OC], FP32)
            nsqrow = const.tile([1, OC], FP32)
            nrmrow = const.tile([1, OC], FP32)
            n1r = const.tile([1, OC], FP32)
            magc = const.tile([1, OC], FP32)
            srow = const.tile([1, OC], FP32)
            scrA = const.tile([128, 1024], FP32)

            for c in range(ncol_limit):
                b2tc = b2tp.tile([RANK, OC], F32R, tag="b2t", name=f"b2tc{c}")
                nc.sync.dma_start(b2tc[:], b2t_d[:, c * OC:(c + 1) * OC].bitcast(F32R))
                ps_n2 = npp.tile([RANK, OC], FP32, tag="np", name=f"psn2_{c}")

                outsb = []
                for h in range(NH):
                    wts = []
                    for j in range(H):
                        ib = h * H + j
                        w_t = wp.tile([128, OC], F32R, tag="w", name=f"w{c}_{ib}")
                        nc.sync.dma_start(w_t[:], wt_d[c, ib].bitcast(F32R))
                        wts.append(w_t)
                    # n2^T = A @ W_col^T partials [rank, OC]: A^T stationary
                    # (16-row weight load hides under the 512-row stream)
                    for j in range(H):
                        ib = h * H + j
                        nc.tensor.matmul(ps_n2[:], aT[:, ib, :], wts[j][:],
                                         start=(ib == 0), stop=(ib == NI - 1))
                    for t in range(NT):
                        ps_m = mp.tile([128, OC], FP32, tag="mp",
                                       name=f"pm{c}_{h}_{t}")
                        for j in range(H):
                            ib = h * H + j
                            nc.tensor.matmul(
                                ps_m[:], xT(ib)[:, t * 128:(t + 1) * 128], wts[j][:],
                                start=(j == 0),
                                stop=(j == H - 1 and h != NH - 1))
                        if h == NH - 1:
                            # rank-16 DoRA term folded into the accumulation
                            nc.tensor.matmul(ps_m[:],
                                             xaT[:, t * 128:(t + 1) * 128],
                                             b2tc[:], start=False, stop=True)
                        if h == 0:
                            o_t = outp.tile([128, OC], FP32, tag="o",
                                            name=f"o{c}_{t}")
                            outsb.append(o_t)
                            nc.vector.tensor_copy(o_t[:], ps_m[:])
                        else:
                            nc.vector.tensor_add(outsb[t][:], outsb[t][:], ps_m[:])

                # n1 for this column's 4 subtiles: ACT Square with row-accum
                # over fp16 W in natural layout (scalar engine is idle)
                for s in range(4):
                    osub = c * 4 + s
                    for k in range(4):
                        wh_t = whp.tile([128, 1024], FP16, tag="wh",
                                        name=f"wh{osub}_{k}")
                        nc.sync.dma_start(
                            wh_t[:], wh_d[:, osub, k * 1024:(k + 1) * 1024])
                        nc.scalar.activation(scrA[:], wh_t[:],
                                             mybir.ActivationFunctionType.Square,
                                             accum_out=n1p[:, k:k + 1])
                    nc.vector.reduce_sum(n1col[:, s:s + 1], n1p[:],
                                         axis=mybir.AxisListType.X)

                # finish norms in row space:
                #   nsq_row = n1_row + ones^T @ ((2*n2T + B2G^T) * B2T)
                ps_t = mp.tile([4, 128], FP32, tag="mp", name=f"pst{c}")
                nc.tensor.transpose(ps_t[:], n1col[:], ident[:])
                nc.vector.tensor_copy(n1row[:], ps_t[:])
                nc.sync.dma_start(n1_d[c:c + 1, :], n1row[:])
                nc.sync.dma_start(n1r[:], n1_d[c:c + 1, :])
                nc.sync.dma_start(magc[:], mag_d[:, c * OC:(c + 1) * OC])
                ps_bg = mp.tile([RANK, OC], FP32, tag="mp", name=f"psbg{c}")
                nc.tensor.matmul(ps_bg[:], g_sb[:], b2tc[:],
                                 start=True, stop=True)
                nc.vector.scalar_tensor_tensor(
                    out=prod2[:], in0=ps_n2[:], scalar=2.0,
                    in1=b2tc[:].bitcast(FP32),
                    op0=mybir.AluOpType.mult, op1=mybir.AluOpType.mult)
                nc.vector.scalar_tensor_tensor(
                    out=prod3[:], in0=ps_bg[:], scalar=1.0,
                    in1=b2tc[:].bitcast(FP32),
                    op0=mybir.AluOpType.mult, op1=mybir.AluOpType.mult)
                nc.vector.tensor_add(prod2[:], prod2[:], prod3[:])
                ps_r = mp.tile([1, OC], FP32, tag="mp", name=f"psr{c}")
                nc.tensor.matmul(ps_r[:], ones16[:], prod2[:],
                                 start=True, stop=True)
                nc.vector.tensor_add(nsqrow[:], ps_r[:], n1r[:])
                nc.scalar.activation(nrmrow[:], nsqrow[:],
                                     mybir.ActivationFunctionType.Sqrt)
                nc.vector.reciprocal(nrmrow[:], nrmrow[:])
                nc.vector.tensor_mul(srow[:], nrmrow[:], magc[:])
                sbc = sbcp.tile([128, OC], FP32, tag="sbc", name=f"sbc{c}")
                nc.sync.dma_start(srow_d[c:c + 1, :], srow[:])
                _sl = srow_d[c:c + 1, :]
                srow_bcast = bass.AP(
                    tensor=_sl.tensor, offset=_sl.offset,
                    ap=[[0, 128], [1, OC]])
                nc.gpsimd.dma_start(sbc[:], srow_bcast)

                for t in range(NT):
                    nc.vector.tensor_mul(outsb[t][:], outsb[t][:], sbc[:])
                    nc.sync.dma_start(
                        out_d[t * 128:(t + 1) * 128, c * OC:(c + 1) * OC],
                        outsb[t][:])

    nc.compile()
    return nc


_PROGRAM = None


def _get_program():
    global _PROGRAM
    if _PROGRAM is None:
        _PROGRAM = _build_program()
    return _PROGRAM


def _prep_inputs(x, weight, lora_a_w, lora_b_w, magnitude):
    xr = _round_f32r(x.reshape(TOK, DIN))
    wr = _round_f32r(weight)
    ar = _round_f32r(lora_a_w)
    b2 = _round_f32r(SCALING * lora_b_w.astype(np.float32))

    wT = np.ascontiguousarray(wr.T)                        # [in, out]
    wt = np.ascontiguousarray(
        wT.reshape(NI, 128, NCOL, OC).transpose(2, 0, 1, 3))
    wh = np.ascontiguousarray(
        wr.astype(np.float16).reshape(32, 128, DIN).transpose(1, 0, 2))
    at = np.ascontiguousarray(ar.T.reshape(NI, 128, RANK).transpose(1, 0, 2))
    b2t = np.ascontiguousarray(b2.T)
    magr = np.ascontiguousarray(
        magnitude.astype(np.float32).reshape(1, DOUT))

    xTfull = xr.T                                           # [in, tok]
    in_maps = []
    for cpu in range(NCORES):
        xs = xTfull[:, cpu * TPC:(cpu + 1) * TPC]
        xt = np.ascontiguousarray(
            xs.reshape(NI, 128, TPC).transpose(1, 0, 2))
        in_maps.append({"xt": xt, "wt": wt, "wh": wh, "at": at,
                        "b2t": b2t, "mag": magr})
    return in_maps


def kernel(x, weight, lora_a_w, lora_b_w, magnitude, _trace=False, **_kw):
    nc = _get_program()
    in_maps = _prep_inputs(x, weight, lora_a_w, lora_b_w, magnitude)
    res = run_bass_kernel_spmd(nc, in_maps, list(range(NCORES)), trace=_trace)
    out = np.concatenate([res.results[c]["out"] for c in range(NCORES)], axis=0)
    if _trace:
        kernel._last_results = res
    return out.reshape(4, 2048, DOUT)



# revision 2
# speedup vs baseline: 1.5071x; 1.5071x over previous
"""DoRA linear layer on 8 TRN2 NeuronCores (bf16 tensor-engine path).

out = (magnitude / ||W + s*B@A||_row) * (x @ (W + s*B@A)^T),  s = alpha/rank = 2.

Identity used: the reference's
    dora_out + base_out = mag_norm_scale * (base_out + s * lora_out)
                        = scale_o * (x @ W^T + s * (x @ A^T) @ B^T)

Structure (per core, data-parallel over tokens, 1024 tok/core):
  - stationary = W^T chunk [128i, 128o], moving = x^T [128i, 512t] (bf16:
    1 col/cycle vs ~2 for fp32r on real HW) -> psum out^T tiles [128o, 512t].
  - n2 = 2*(W @ A^T) rides the same stationary as extra 16-col matmuls into
    a [128o, 16] psum; B2@G accumulates into the same psum, so the row norm
    finishes as ONE fused multiply-accumulate against B2 (natural layout)
    plus n1 = rowsum(W^2) from an fp16 W copy on the scalar engine.
  - All norm/scale math lives in o-partition space: scale is a [128,1]
    per-partition broadcast, no transposes, no DRAM round-trip.
  - out^T written bf16; host transposes/casts back to [tok, out] fp32.
"""

import sys

sys.path.insert(0, "/opt/trn_rl_repo")

import numpy as np
import ml_dtypes

import concourse.bass as bass  # noqa: F401  (import keeps bass registered)
from concourse import bacc
import concourse.mybir as mybir
from concourse.tile import TileContext
from concourse.bass_utils import run_bass_kernel_spmd

FP32 = mybir.dt.float32
BF16 = mybir.dt.bfloat16
FP16 = mybir.dt.float16

BF = ml_dtypes.bfloat16

NCORES = 8
TOK = 8192          # 4 * 2048 tokens
TPC = TOK // NCORES  # 1024 tokens per core
DIN = 4096
DOUT = 4096
RANK = 16
SCALING = 32.0 / 16

NI = DIN // 128      # 32 contraction blocks
NOC = DOUT // 128    # 32 output chunks of 128
NXQ = 4              # x DMA split (ib-groups)


def _build_program():
    nc = bacc.Bacc("TRN2", target_bir_lowering=False, debug=False,
                   num_devices=NCORES)

    xt_d = nc.dram_tensor("xt", [128, NI, TPC], BF16, kind="ExternalInput")
    wt_d = nc.dram_tensor("wt", [NOC, 128, NI, 128], BF16, kind="ExternalInput")
    wh_d = nc.dram_tensor("wh", [128, NOC, DIN], FP16, kind="ExternalInput")
    at_d = nc.dram_tensor("at", [128, NI, RANK], BF16, kind="ExternalInput")
    a2t_d = nc.dram_tensor("a2t", [128, NI, RANK], BF16, kind="ExternalInput")
    b2t_d = nc.dram_tensor("b2t", [RANK, DOUT], BF16, kind="ExternalInput")
    b2n_d = nc.dram_tensor("b2n", [128, NOC, RANK], BF16, kind="ExternalInput")
    mag_d = nc.dram_tensor("mag", [128, NOC], FP32, kind="ExternalInput")
    out_d = nc.dram_tensor("out", [DOUT, TPC], BF16, kind="ExternalOutput")

    with TileContext(nc) as tc:
        with (
            tc.tile_pool(name="const", bufs=1) as const,
            tc.tile_pool(name="xtp", bufs=1) as xtp,
            tc.tile_pool(name="wtp", bufs=4) as wtp,
            tc.tile_pool(name="whp", bufs=3) as whp,
            tc.tile_pool(name="outp", bufs=6) as outp,
            tc.tile_pool(name="scl", bufs=4) as scl,
            tc.tile_pool(name="mp", bufs=4, space="PSUM") as mp,
            tc.tile_pool(name="np2", bufs=2, space="PSUM") as np2,
        ):
            aT = const.tile([128, NI, RANK], BF16)
            nc.sync.dma_start(aT[:], at_d[:])
            a2T = const.tile([128, NI, RANK], BF16)
            nc.sync.dma_start(a2T[:], a2t_d[:])
            b2t = const.tile([RANK, DOUT], BF16)
            nc.sync.dma_start(b2t[:], b2t_d[:])
            b2n = const.tile([128, NOC, RANK], BF16)
            nc.sync.dma_start(b2n[:], b2n_d[:])
            mag = const.tile([128, NOC], FP32)
            nc.sync.dma_start(mag[:], mag_d[:])

            # resident x^T [i_part, i_blk, tok], split DMAs so early i-blocks
            # unblock chunk 0 before the whole 8 MiB lands
            QI = NI // NXQ
            xTq = []
            for q in range(NXQ):
                xq = xtp.tile([128, QI, TPC], BF16, name=f"xTq{q}")
                nc.sync.dma_start(xq[:], xt_d[:, q * QI:(q + 1) * QI, :])
                xTq.append(xq)

            def xT(ib):
                return xTq[ib // QI][:, ib % QI, :]

            # G = A @ A^T  [rank, rank]
            ps_g = mp.tile([RANK, RANK], FP32, tag="mp", name="psg")
            for ib in range(NI):
                nc.tensor.matmul(ps_g[:], aT[:, ib, :], aT[:, ib, :],
                                 start=(ib == 0), stop=(ib == NI - 1))
            g_sb = const.tile([RANK, RANK], BF16)
            nc.vector.tensor_copy(g_sb[:], ps_g[:])

            # xa^T = (x @ A^T)^T  [rank, tok]
            xaT = const.tile([RANK, TPC], BF16)
            for q in range(2):
                ps_xa = mp.tile([RANK, 512], FP32, tag="mp", name=f"psxa{q}")
                for ib in range(NI):
                    nc.tensor.matmul(
                        ps_xa[:], aT[:, ib, :], xT(ib)[:, q * 512:(q + 1) * 512],
                        start=(ib == 0), stop=(ib == NI - 1))
                nc.vector.tensor_copy(xaT[:, q * 512:(q + 1) * 512], ps_xa[:])

            for c in range(NOC):
                wt_c = wtp.tile([128, NI, 128], BF16, tag="w", name=f"w{c}")
                nc.sync.dma_start(wt_c[:], wt_d[c])
                wh_c = whp.tile([128, DIN], FP16, tag="wh", name=f"wh{c}")
                nc.sync.dma_start(wh_c[:], wh_d[:, c, :])

                ps0 = mp.tile([128, 512], FP32, tag="mp", name=f"ps0_{c}")
                ps1 = mp.tile([128, 512], FP32, tag="mp", name=f"ps1_{c}")
                pn2 = np2.tile([128, RANK], FP32, tag="np2", name=f"pn2_{c}")
                for ib in range(NI):
                    w = wt_c[:, ib, :]
                    nc.tensor.matmul(ps0[:], w, xT(ib)[:, 0:512],
                                     start=(ib == 0), stop=False)
                    nc.tensor.matmul(pn2[:], w, a2T[:, ib, :],
                                     start=(ib == 0), stop=False)
                    nc.tensor.matmul(ps1[:], w, xT(ib)[:, 512:1024],
                                     start=(ib == 0), stop=False)
                b2c = b2t[:, c * 128:(c + 1) * 128]
                # + B2@G into the n2 psum: row norm becomes one fused reduce
                nc.tensor.matmul(pn2[:], b2c, g_sb[:], start=False, stop=True)
                # rank-16 DoRA term folded into the out accumulation
                nc.tensor.matmul(ps0[:], b2c, xaT[:, 0:512],
                                 start=False, stop=True)
                nc.tensor.matmul(ps1[:], b2c, xaT[:, 512:1024],
                                 start=False, stop=True)

                # n1 = rowsum(W^2) on the scalar engine (fp16 natural layout)
                n1p = scl.tile([128, 4], FP32, tag="n1p", name=f"n1p{c}")
                for k in range(4):
                    nc.scalar.activation(
                        scl.tile([128, 1024], FP32, tag="sqw",
                                 name=f"sqw{c}_{k}")[:],
                        wh_c[:, k * 1024:(k + 1) * 1024],
                        mybir.ActivationFunctionType.Square,
                        accum_out=n1p[:, k:k + 1])
                n1c = scl.tile([128, 1], FP32, tag="n1c", name=f"n1c{c}")
                nc.vector.reduce_sum(n1c[:], n1p[:], axis=mybir.AxisListType.X)

                # cross + lowrank norm terms: sum_r pn2[o,r] * B2[o,r]
                cr = scl.tile([128, 1], FP32, tag="cr", name=f"cr{c}")
                nc.vector.scalar_tensor_tensor(
                    out=scl.tile([128, RANK], FP32, tag="scr",
                                 name=f"scr{c}")[:],
                    in0=pn2[:], scalar=1.0, in1=b2n[:, c, :],
                    op0=mybir.AluOpType.mult, op1=mybir.AluOpType.mult,
                    accum_out=cr[:])
                nsq = scl.tile([128, 1], FP32, tag="nsq", name=f"nsq{c}")
                nc.vector.tensor_add(nsq[:], cr[:], n1c[:])
                nrm = scl.tile([128, 1], FP32, tag="nrm", name=f"nrm{c}")
                nc.scalar.activation(nrm[:], nsq[:],
                                     mybir.ActivationFunctionType.Sqrt)
                nc.vector.reciprocal(nrm[:], nrm[:])
                sc = scl.tile([128, 1], FP32, tag="sc", name=f"sc{c}")
                nc.vector.tensor_mul(sc[:], nrm[:], mag[:, c:c + 1])

                o0 = outp.tile([128, 512], BF16, tag="o", name=f"o0_{c}")
                nc.vector.tensor_scalar_mul(o0[:], ps0[:], sc[:])
                nc.sync.dma_start(out_d[c * 128:(c + 1) * 128, 0:512], o0[:])
                o1 = outp.tile([128, 512], BF16, tag="o", name=f"o1_{c}")
                nc.vector.tensor_scalar_mul(o1[:], ps1[:], sc[:])
                nc.sync.dma_start(out_d[c * 128:(c + 1) * 128, 512:1024], o1[:])

    nc.compile()
    return nc


_PROGRAM = None


def _get_program():
    global _PROGRAM
    if _PROGRAM is None:
        _PROGRAM = _build_program()
    return _PROGRAM


def _prep_inputs(x, weight, lora_a_w, lora_b_w, magnitude):
    w32 = weight.astype(np.float32)
    wb = w32.astype(BF)
    wt = np.ascontiguousarray(
        wb.T.reshape(NI, 128, NOC, 128).transpose(2, 1, 0, 3))
    wh = np.ascontiguousarray(
        w32.astype(np.float16).reshape(NOC, 128, DIN).transpose(1, 0, 2))
    a32 = lora_a_w.astype(np.float32)
    at = np.ascontiguousarray(
        a32.astype(BF).T.reshape(NI, 128, RANK).transpose(1, 0, 2))
    a2t = np.ascontiguousarray(
        (2.0 * a32).astype(BF).T.reshape(NI, 128, RANK).transpose(1, 0, 2))
    b2 = (SCALING * lora_b_w.astype(np.float32)).astype(BF)
    b2t = np.ascontiguousarray(b2.T)
    b2n = np.ascontiguousarray(
        b2.reshape(NOC, 128, RANK).transpose(1, 0, 2))
    magr = np.ascontiguousarray(
        magnitude.astype(np.float32).reshape(NOC, 128).T)

    xb = x.reshape(TOK, DIN).astype(BF)
    in_maps = []
    for cpu in range(NCORES):
        xs = xb[cpu * TPC:(cpu + 1) * TPC].T
        xt = np.ascontiguousarray(
            xs.reshape(NI, 128, TPC).transpose(1, 0, 2))
        in_maps.append({"xt": xt, "wt": wt, "wh": wh, "at": at, "a2t": a2t,
                        "b2t": b2t, "b2n": b2n, "mag": magr})
    return in_maps


def kernel(x, weight, lora_a_w, lora_b_w, magnitude, _trace=False, **_kw):
    nc = _get_program()
    in_maps = _prep_inputs(x, weight, lora_a_w, lora_b_w, magnitude)
    res = run_bass_kernel_spmd(nc, in_maps, list(range(NCORES)), trace=_trace)
    out = np.empty((TOK, DOUT), dtype=np.float32)
    for c in range(NCORES):
        out[c * TPC:(c + 1) * TPC] = res.results[c]["out"].T.astype(np.float32)
    if _trace:
        kernel._last_results = res
    return out.reshape(4, 2048, DOUT)


# revision 5
# speedup vs baseline: 1.5139x; 1.0045x over previous
"""DoRA linear layer on 8 TRN2 NeuronCores (bf16 tensor-engine path).

out = (magnitude / ||W + s*B@A||_row) * (x @ (W + s*B@A)^T),  s = alpha/rank = 2.

Identity used: the reference's
    dora_out + base_out = mag_norm_scale * (base_out + s * lora_out)
                        = scale_o * (x @ W^T + s * (x @ A^T) @ B^T)

Structure (per core, data-parallel over tokens, 1024 tok/core):
  - stationary = W^T chunk [128i, 128o], moving = x^T [128i, 512t] (bf16:
    1 col/cycle vs ~2 for fp32r on real HW) -> psum out^T tiles [128o, 512t].
  - n2 = 2*(W @ A^T) rides the same stationary as extra 16-col matmuls into
    a [128o, 16] psum; B2@G accumulates into the same psum, so the row norm
    finishes as ONE fused multiply-accumulate against B2 (natural layout)
    plus n1 = rowsum(W^2) from an fp16 W copy on the scalar engine.
  - All norm/scale math lives in o-partition space: scale is a [128,1]
    per-partition broadcast, no transposes, no DRAM round-trip.
  - out^T written bf16; host transposes/casts back to [tok, out] fp32.
"""

import sys

sys.path.insert(0, "/opt/trn_rl_repo")

import numpy as np
import ml_dtypes

import concourse.bass as bass  # noqa: F401  (import keeps bass registered)
from concourse import bacc
import concourse.mybir as mybir
from concourse.tile import TileContext
from concourse.bass_utils import run_bass_kernel_spmd

FP32 = mybir.dt.float32
BF16 = mybir.dt.bfloat16
FP16 = mybir.dt.float16

BF = ml_dtypes.bfloat16

NCORES = 8
TOK = 8192          # 4 * 2048 tokens
TPC = TOK // NCORES  # 1024 tokens per core
DIN = 4096
DOUT = 4096
RANK = 16
SCALING = 32.0 / 16

NI = DIN // 128      # 32 contraction blocks
NOC = DOUT // 128    # 32 output chunks of 128
NXQ = 4              # x DMA split (ib-groups)


def _build_program():
    nc = bacc.Bacc("TRN2", target_bir_lowering=False, debug=False,
                   num_devices=NCORES)

    xt_d = nc.dram_tensor("xt", [128, NI, TPC], BF16, kind="ExternalInput")
    wt_d = nc.dram_tensor("wt", [NOC, 128, NI, 128], BF16, kind="ExternalInput")
    wh_d = nc.dram_tensor("wh", [128, NOC, DIN], FP16, kind="ExternalInput")
    at_d = nc.dram_tensor("at", [128, NI, RANK], BF16, kind="ExternalInput")
    a2t_d = nc.dram_tensor("a2t", [128, NI, RANK], BF16, kind="ExternalInput")
    b2t_d = nc.dram_tensor("b2t", [RANK, DOUT], BF16, kind="ExternalInput")
    b2n_d = nc.dram_tensor("b2n", [128, NOC, RANK], BF16, kind="ExternalInput")
    mag_d = nc.dram_tensor("mag", [128, NOC], FP32, kind="ExternalInput")
    out_d = nc.dram_tensor("out", [DOUT, TPC], BF16, kind="ExternalOutput")

    with TileContext(nc) as tc:
        with (
            tc.tile_pool(name="const", bufs=1) as const,
            tc.tile_pool(name="xtp", bufs=1) as xtp,
            tc.tile_pool(name="wtp", bufs=4) as wtp,
            tc.tile_pool(name="whp", bufs=3) as whp,
            tc.tile_pool(name="outp", bufs=6) as outp,
            tc.tile_pool(name="scl", bufs=4) as scl,
            tc.tile_pool(name="mp", bufs=4, space="PSUM") as mp,
            tc.tile_pool(name="np2", bufs=2, space="PSUM") as np2,
            tc.tile_pool(name="xap", bufs=2, space="PSUM") as xap,
        ):
            aT = const.tile([128, NI, RANK], BF16)
            nc.sync.dma_start(aT[:], at_d[:])
            a2T = const.tile([128, NI, RANK], BF16)
            nc.sync.dma_start(a2T[:], a2t_d[:])

            # resident x^T [i_part, i_blk, tok], split DMAs so early i-blocks
            # unblock chunk 0 before the whole 8 MiB lands
            QI = NI // NXQ
            xTq = []
            for q in range(NXQ):
                xq = xtp.tile([128, QI, TPC], BF16, name=f"xTq{q}")
                for h in range(2):
                    nc.sync.dma_start(
                        xq[:, h * (QI // 2):(h + 1) * (QI // 2), :],
                        xt_d[:, q * QI + h * (QI // 2):
                             q * QI + (h + 1) * (QI // 2), :])
                xTq.append(xq)

            b2t = const.tile([RANK, DOUT], BF16)
            nc.sync.dma_start(b2t[:], b2t_d[:])
            b2n = const.tile([128, NOC, RANK], BF16)
            nc.sync.dma_start(b2n[:], b2n_d[:])
            mag = const.tile([128, NOC], FP32)
            nc.sync.dma_start(mag[:], mag_d[:])

            def xT(ib):
                return xTq[ib // QI][:, ib % QI, :]

            # G = A @ A^T  [rank, rank] — needs only aT, runs immediately
            ps_g = xap.tile([RANK, RANK], FP32, tag="xap", name="psg")
            for ib in range(NI):
                nc.tensor.matmul(ps_g[:], aT[:, ib, :], aT[:, ib, :],
                                 start=(ib == 0), stop=(ib == NI - 1))
            g_sb = const.tile([RANK, RANK], BF16)
            nc.vector.tensor_copy(g_sb[:], ps_g[:])

            # xa^T = (x @ A^T)^T [rank, tok]: accumulated inside chunk 0's
            # i-loop so the PE races the x DMA instead of stalling on it
            xaT = const.tile([RANK, TPC], BF16)
            ps_xa = [xap.tile([RANK, 512], FP32, tag="xap", name=f"psxa{q}")
                     for q in range(2)]

            for c in range(NOC):
                wt_c = wtp.tile([128, NI, 128], BF16, tag="w", name=f"w{c}")
                nc.sync.dma_start(wt_c[:], wt_d[c])
                wh_c = whp.tile([128, DIN], FP16, tag="wh", name=f"wh{c}")
                nc.sync.dma_start(wh_c[:], wh_d[:, c, :])

                # n1 = rowsum(W^2) on the scalar engine (fp16 natural layout);
                # emitted before the i-loop so the last chunk's ACT work is
                # not exposed in the tail
                n1p = scl.tile([128, 4], FP32, tag="n1p", name=f"n1p{c}")
                for k in range(4):
                    nc.scalar.activation(
                        scl.tile([128, 1024], FP32, tag="sqw",
                                 name=f"sqw{c}_{k}")[:],
                        wh_c[:, k * 1024:(k + 1) * 1024],
                        mybir.ActivationFunctionType.Square,
                        accum_out=n1p[:, k:k + 1])
                n1c = scl.tile([128, 1], FP32, tag="n1c", name=f"n1c{c}")
                nc.vector.reduce_sum(n1c[:], n1p[:], axis=mybir.AxisListType.X)

                ps0 = mp.tile([128, 512], FP32, tag="mp", name=f"ps0_{c}")
                ps1 = mp.tile([128, 512], FP32, tag="mp", name=f"ps1_{c}")
                pn2 = np2.tile([128, RANK], FP32, tag="np2", name=f"pn2_{c}")
                for ib in range(NI):
                    w = wt_c[:, ib, :]
                    if c == 0:
                        nc.tensor.matmul(ps_xa[0][:], aT[:, ib, :],
                                         xT(ib)[:, 0:512],
                                         start=(ib == 0), stop=(ib == NI - 1))
                        nc.tensor.matmul(ps_xa[1][:], aT[:, ib, :],
                                         xT(ib)[:, 512:1024],
                                         start=(ib == 0), stop=(ib == NI - 1))
                    nc.tensor.matmul(ps0[:], w, xT(ib)[:, 0:512],
                                     start=(ib == 0), stop=False)
                    nc.tensor.matmul(pn2[:], w, a2T[:, ib, :],
                                     start=(ib == 0), stop=False)
                    nc.tensor.matmul(ps1[:], w, xT(ib)[:, 512:1024],
                                     start=(ib == 0), stop=False)
                if c == 0:
                    for q in range(2):
                        nc.vector.tensor_copy(
                            xaT[:, q * 512:(q + 1) * 512], ps_xa[q][:])
                b2c = b2t[:, c * 128:(c + 1) * 128]
                # + B2@G into the n2 psum: row norm becomes one fused reduce
                nc.tensor.matmul(pn2[:], b2c, g_sb[:], start=False, stop=True)
                # rank-16 DoRA term folded into the out accumulation
                nc.tensor.matmul(ps0[:], b2c, xaT[:, 0:512],
                                 start=False, stop=True)
                nc.tensor.matmul(ps1[:], b2c, xaT[:, 512:1024],
                                 start=False, stop=True)

                # cross + lowrank norm terms: sum_r pn2[o,r] * B2[o,r]
                cr = scl.tile([128, 1], FP32, tag="cr", name=f"cr{c}")
                nc.vector.scalar_tensor_tensor(
                    out=scl.tile([128, RANK], FP32, tag="scr",
                                 name=f"scr{c}")[:],
                    in0=pn2[:], scalar=1.0, in1=b2n[:, c, :],
                    op0=mybir.AluOpType.mult, op1=mybir.AluOpType.mult,
                    accum_out=cr[:])
                nsq = scl.tile([128, 1], FP32, tag="nsq", name=f"nsq{c}")
                nc.vector.tensor_add(nsq[:], cr[:], n1c[:])
                nrm = scl.tile([128, 1], FP32, tag="nrm", name=f"nrm{c}")
                nc.scalar.activation(nrm[:], nsq[:],
                                     mybir.ActivationFunctionType.Sqrt)
                nc.vector.reciprocal(nrm[:], nrm[:])
                sc = scl.tile([128, 1], FP32, tag="sc", name=f"sc{c}")
                nc.vector.tensor_mul(sc[:], nrm[:], mag[:, c:c + 1])

                o0 = outp.tile([128, 512], BF16, tag="o", name=f"o0_{c}")
                nc.vector.tensor_scalar_mul(o0[:], ps0[:], sc[:])
                nc.sync.dma_start(out_d[c * 128:(c + 1) * 128, 0:512], o0[:])
                o1 = outp.tile([128, 512], BF16, tag="o", name=f"o1_{c}")
                nc.vector.tensor_scalar_mul(o1[:], ps1[:], sc[:])
                nc.sync.dma_start(out_d[c * 128:(c + 1) * 128, 512:1024], o1[:])

    nc.compile()
    return nc


_PROGRAM = None


def _get_program():
    global _PROGRAM
    if _PROGRAM is None:
        _PROGRAM = _build_program()
    return _PROGRAM


def _prep_inputs(x, weight, lora_a_w, lora_b_w, magnitude):
    w32 = weight.astype(np.float32)
    wb = w32.astype(BF)
    wt = np.ascontiguousarray(
        wb.T.reshape(NI, 128, NOC, 128).transpose(2, 1, 0, 3))
    wh = np.ascontiguousarray(
        w32.astype(np.float16).reshape(NOC, 128, DIN).transpose(1, 0, 2))
    a32 = lora_a_w.astype(np.float32)
    at = np.ascontiguousarray(
        a32.astype(BF).T.reshape(NI, 128, RANK).transpose(1, 0, 2))
    a2t = np.ascontiguousarray(
        (2.0 * a32).astype(BF).T.reshape(NI, 128, RANK).transpose(1, 0, 2))
    b2 = (SCALING * lora_b_w.astype(np.float32)).astype(BF)
    b2t = np.ascontiguousarray(b2.T)
    b2n = np.ascontiguousarray(
        b2.reshape(NOC, 128, RANK).transpose(1, 0, 2))
    magr = np.ascontiguousarray(
        magnitude.astype(np.float32).reshape(NOC, 128).T)

    xb = x.reshape(TOK, DIN).astype(BF)
    in_maps = []
    for cpu in range(NCORES):
        xs = xb[cpu * TPC:(cpu + 1) * TPC].T
        xt = np.ascontiguousarray(
            xs.reshape(NI, 128, TPC).transpose(1, 0, 2))
        in_maps.append({"xt": xt, "wt": wt, "wh": wh, "at": at, "a2t": a2t,
                        "b2t": b2t, "b2n": b2n, "mag": magr})
    return in_maps


def kernel(x, weight, lora_a_w, lora_b_w, magnitude, _trace=False, **_kw):
    nc = _get_program()
    in_maps = _prep_inputs(x, weight, lora_a_w, lora_b_w, magnitude)
    res = run_bass_kernel_spmd(nc, in_maps, list(range(NCORES)), trace=_trace)
    out = np.empty((TOK, DOUT), dtype=np.float32)
    for c in range(NCORES):
        out[c * TPC:(c + 1) * TPC] = res.results[c]["out"].T.astype(np.float32)
    if _trace:
        kernel._last_results = res
    return out.reshape(4, 2048, DOUT)


# revision 11
# speedup vs baseline: 1.5306x; 1.0111x over previous
"""DoRA linear layer on 8 TRN2 NeuronCores (bf16 tensor-engine path).

out = (magnitude / ||W + s*B@A||_row) * (x @ (W + s*B@A)^T),  s = alpha/rank = 2.

Identity used: the reference's
    dora_out + base_out = mag_norm_scale * (base_out + s * lora_out)
                        = scale_o * (x @ W^T + s * (x @ A^T) @ B^T)

Structure (per core, data-parallel over tokens, 1024 tok/core):
  - stationary = W^T chunk [128i, 128o], moving = x^T [128i, 512t] (bf16:
    1 col/cycle vs ~2 for fp32r on real HW) -> psum out^T tiles [128o, 512t].
  - n2 = 2*(W @ A^T) rides the same stationary as extra 16-col matmuls into
    a [128o, 16] psum; B2@G accumulates into the same psum, so the row norm
    finishes as ONE fused multiply-accumulate against B2 (natural layout)
    plus n1 = rowsum(W^2) from an fp16 W copy on the scalar engine.
  - All norm/scale math lives in o-partition space: scale is a [128,1]
    per-partition broadcast, no transposes, no DRAM round-trip.
  - out^T written bf16; host transposes/casts back to [tok, out] fp32.
"""

import sys

sys.path.insert(0, "/opt/trn_rl_repo")

import numpy as np
import ml_dtypes

import concourse.bass as bass  # noqa: F401  (import keeps bass registered)
from concourse import bacc
import concourse.mybir as mybir
from concourse.tile import TileContext
from concourse.bass_utils import run_bass_kernel_spmd

FP32 = mybir.dt.float32
BF16 = mybir.dt.bfloat16
FP16 = mybir.dt.float16

BF = ml_dtypes.bfloat16

NCORES = 8
TOK = 8192          # 4 * 2048 tokens
TPC = TOK // NCORES  # 1024 tokens per core
DIN = 4096
DOUT = 4096
RANK = 16
SCALING = 32.0 / 16

NI = DIN // 128      # 32 contraction blocks
NOC = DOUT // 128    # 32 output chunks of 128
NXQ = 4              # x DMA split (ib-groups)


def _build_program():
    nc = bacc.Bacc("TRN2", target_bir_lowering=False, debug=False,
                   num_devices=NCORES)

    xt_d = nc.dram_tensor("xt", [128, NI, TPC], BF16, kind="ExternalInput")
    wt_d = nc.dram_tensor("wt", [NOC, 128, NI, 128], BF16, kind="ExternalInput")
    wh_d = nc.dram_tensor("wh", [128, NOC, DIN], FP16, kind="ExternalInput")
    at_d = nc.dram_tensor("at", [128, NI, RANK], BF16, kind="ExternalInput")
    a2t_d = nc.dram_tensor("a2t", [128, NI, RANK], BF16, kind="ExternalInput")
    b2t_d = nc.dram_tensor("b2t", [RANK, DOUT], BF16, kind="ExternalInput")
    b2n_d = nc.dram_tensor("b2n", [128, NOC, RANK], BF16, kind="ExternalInput")
    mag_d = nc.dram_tensor("mag", [128, NOC], FP32, kind="ExternalInput")
    out_d = nc.dram_tensor("out", [DOUT, TPC], BF16, kind="ExternalOutput")

    with TileContext(nc) as tc:
        with (
            tc.tile_pool(name="const", bufs=1) as const,
            tc.tile_pool(name="xtp", bufs=1) as xtp,
            tc.tile_pool(name="wtp", bufs=4) as wtp,
            tc.tile_pool(name="whp", bufs=3) as whp,
            tc.tile_pool(name="outp", bufs=6) as outp,
            tc.tile_pool(name="scl", bufs=4) as scl,
            tc.tile_pool(name="mp", bufs=4, space="PSUM") as mp,
            tc.tile_pool(name="np2", bufs=2, space="PSUM") as np2,
            tc.tile_pool(name="xap", bufs=2, space="PSUM") as xap,
        ):
            aT = const.tile([128, NI, RANK], BF16)
            nc.sync.dma_start(aT[:], at_d[:])
            a2T = const.tile([128, NI, RANK], BF16)
            nc.sync.dma_start(a2T[:], a2t_d[:])

            # resident x^T [i_part, i_blk, tok]: one DMA per i-block — a
            # single DMA queue moves only ~23 GB/s, so landing time is set
            # by how many queues a tensor is spread across
            QI = NI // NXQ
            xTq = [xtp.tile([128, QI, TPC], BF16, name=f"xTq{q}")
                   for q in range(NXQ)]
            b2t = const.tile([RANK, DOUT], BF16)
            b2n = const.tile([128, NOC, RANK], BF16)
            mag = const.tile([128, NOC], FP32)

            def xT(ib):
                return xTq[ib // QI][:, ib % QI, :]

            def dma_xq(q):
                for j in range(QI):
                    nc.sync.dma_start(xTq[q][:, j, :], xt_d[:, q * QI + j, :])

            dma_xq(0)

            def chunk_tiles(c, with_wh=True):
                wt_c = wtp.tile([128, NI, 128], BF16, tag="w", name=f"w{c}")
                for h in range(4):
                    nc.sync.dma_start(wt_c[:, h * 8:(h + 1) * 8, :],
                                      wt_d[c, :, h * 8:(h + 1) * 8, :])
                t = {"c": c, "wt": wt_c}
                if with_wh:
                    chunk_wh(t)
                return t

            def chunk_wh(t):
                c = t["c"]
                # n1 = rowsum(W^2) on the scalar engine (fp16 natural
                # layout); one DMA piece per ACT op
                wh_c = whp.tile([128, DIN], FP16, tag="wh", name=f"wh{c}")
                n1p = scl.tile([128, 4], FP32, tag="n1p", name=f"n1p{c}")
                for k in range(4):
                    nc.sync.dma_start(wh_c[:, k * 1024:(k + 1) * 1024],
                                      wh_d[:, c, k * 1024:(k + 1) * 1024])
                    nc.scalar.activation(
                        scl.tile([128, 1024], FP32, tag="sqw",
                                 name=f"sqw{c}_{k}")[:],
                        wh_c[:, k * 1024:(k + 1) * 1024],
                        mybir.ActivationFunctionType.Square,
                        accum_out=n1p[:, k:k + 1])
                n1c = scl.tile([128, 1], FP32, tag="n1c", name=f"n1c{c}")
                nc.vector.reduce_sum(n1c[:], n1p[:], axis=mybir.AxisListType.X)
                t["n1c"] = n1c

            def chunk_psums(t):
                c = t["c"]
                t["ps0"] = mp.tile([128, 512], FP32, tag="mp", name=f"ps0_{c}")
                t["ps1"] = mp.tile([128, 512], FP32, tag="mp", name=f"ps1_{c}")
                t["pn2"] = np2.tile([128, RANK], FP32, tag="np2",
                                    name=f"pn2_{c}")

            def chunk_ib(t, ib):
                w = t["wt"][:, ib, :]
                nc.tensor.matmul(t["ps0"][:], w, xT(ib)[:, 0:512],
                                 start=(ib == 0), stop=False)
                nc.tensor.matmul(t["pn2"][:], w, a2T[:, ib, :],
                                 start=(ib == 0), stop=False)
                nc.tensor.matmul(t["ps1"][:], w, xT(ib)[:, 512:1024],
                                 start=(ib == 0), stop=False)

            def chunk_finish(t):
                c = t["c"]
                ps0, ps1, pn2 = t["ps0"], t["ps1"], t["pn2"]
                b2c = b2t[:, c * 128:(c + 1) * 128]
                # + B2@G into the n2 psum: row norm finishes as one fused
                # multiply-accumulate against B2
                nc.tensor.matmul(pn2[:], b2c, g_sb[:], start=False, stop=True)
                # rank-16 DoRA term folded into the out accumulation
                nc.tensor.matmul(ps0[:], b2c, xaT[:, 0:512],
                                 start=False, stop=True)
                nc.tensor.matmul(ps1[:], b2c, xaT[:, 512:1024],
                                 start=False, stop=True)

                # cross + lowrank norm terms: sum_r pn2[o,r] * B2[o,r]
                cr = scl.tile([128, 1], FP32, tag="cr", name=f"cr{c}")
                nc.vector.scalar_tensor_tensor(
                    out=scl.tile([128, RANK], FP32, tag="scr",
                                 name=f"scr{c}")[:],
                    in0=pn2[:], scalar=1.0, in1=b2n[:, c, :],
                    op0=mybir.AluOpType.mult, op1=mybir.AluOpType.mult,
                    accum_out=cr[:])
                nsq = scl.tile([128, 1], FP32, tag="nsq", name=f"nsq{c}")
                nc.vector.tensor_add(nsq[:], cr[:], t["n1c"][:])
                nrm = scl.tile([128, 1], FP32, tag="nrm", name=f"nrm{c}")
                nc.scalar.activation(nrm[:], nsq[:],
                                     mybir.ActivationFunctionType.Sqrt)
                nc.vector.reciprocal(nrm[:], nrm[:])
                sc = scl.tile([128, 1], FP32, tag="sc", name=f"sc{c}")
                nc.vector.tensor_mul(sc[:], nrm[:], mag[:, c:c + 1])

                o0 = outp.tile([128, 512], BF16, tag="o", name=f"o0_{c}")
                nc.vector.tensor_scalar_mul(o0[:], ps0[:], sc[:])
                nc.sync.dma_start(out_d[c * 128:(c + 1) * 128, 0:512], o0[:])
                o1 = outp.tile([128, 512], BF16, tag="o", name=f"o1_{c}")
                nc.vector.tensor_scalar_mul(o1[:], ps1[:], sc[:])
                nc.sync.dma_start(out_d[c * 128:(c + 1) * 128, 512:1024],
                                  o1[:])

            # ---- chunks 0+1, interleaved per x i-block group to race the
            # x DMA; xa accumulates alongside ----
            t0 = chunk_tiles(0, with_wh=False)
            t1 = chunk_tiles(1, with_wh=False)
            chunk_psums(t0)
            chunk_psums(t1)

            # G = A @ A^T  [rank, rank] — needs only aT, runs immediately
            # (borrows an np2 bank; drained well before pn2_1 needs it)
            ps_g = np2.tile([RANK, RANK], FP32, tag="np2", name="psg")
            for ib in range(NI):
                nc.tensor.matmul(ps_g[:], aT[:, ib, :], aT[:, ib, :],
                                 start=(ib == 0), stop=(ib == NI - 1))
            g_sb = const.tile([RANK, RANK], BF16)
            nc.vector.tensor_copy(g_sb[:], ps_g[:])

            # xa^T = (x @ A^T)^T [rank, tok]
            xaT = const.tile([RANK, TPC], BF16)
            ps_xa = [xap.tile([RANK, 512], FP32, tag="xap", name=f"psxa{q}")
                     for q in range(2)]

            for q in range(NXQ):
                if q + 1 < NXQ:
                    dma_xq(q + 1)
                if q == 1:
                    chunk_wh(t0)
                    nc.sync.dma_start(b2t[:], b2t_d[:])
                if q == 2:
                    chunk_wh(t1)
                    nc.sync.dma_start(b2n[:], b2n_d[:])
                    nc.sync.dma_start(mag[:], mag_d[:])
                for j in range(QI):
                    ib = q * QI + j
                    nc.tensor.matmul(ps_xa[0][:], aT[:, ib, :],
                                     xT(ib)[:, 0:512],
                                     start=(ib == 0), stop=(ib == NI - 1))
                    nc.tensor.matmul(ps_xa[1][:], aT[:, ib, :],
                                     xT(ib)[:, 512:1024],
                                     start=(ib == 0), stop=(ib == NI - 1))
                    chunk_ib(t0, ib)
                    chunk_ib(t1, ib)
            for q in range(2):
                nc.vector.tensor_copy(xaT[:, q * 512:(q + 1) * 512],
                                      ps_xa[q][:])
            chunk_finish(t0)
            chunk_finish(t1)

            # ---- steady-state chunks ----
            for c in range(2, NOC):
                t = chunk_tiles(c)
                chunk_psums(t)
                for ib in range(NI):
                    chunk_ib(t, ib)
                chunk_finish(t)

    nc.compile()
    return nc


_PROGRAM = None


def _get_program():
    global _PROGRAM
    if _PROGRAM is None:
        _PROGRAM = _build_program()
    return _PROGRAM


def _prep_inputs(x, weight, lora_a_w, lora_b_w, magnitude):
    w32 = weight.astype(np.float32)
    wb = w32.astype(BF)
    wt = np.ascontiguousarray(
        wb.T.reshape(NI, 128, NOC, 128).transpose(2, 1, 0, 3))
    wh = np.ascontiguousarray(
        w32.astype(np.float16).reshape(NOC, 128, DIN).transpose(1, 0, 2))
    a32 = lora_a_w.astype(np.float32)
    at = np.ascontiguousarray(
        a32.astype(BF).T.reshape(NI, 128, RANK).transpose(1, 0, 2))
    a2t = np.ascontiguousarray(
        (2.0 * a32).astype(BF).T.reshape(NI, 128, RANK).transpose(1, 0, 2))
    b2 = (SCALING * lora_b_w.astype(np.float32)).astype(BF)
    b2t = np.ascontiguousarray(b2.T)
    b2n = np.ascontiguousarray(
        b2.reshape(NOC, 128, RANK).transpose(1, 0, 2))
    magr = np.ascontiguousarray(
        magnitude.astype(np.float32).reshape(NOC, 128).T)

    xb = x.reshape(TOK, DIN).astype(BF)
    in_maps = []
    for cpu in range(NCORES):
        xs = xb[cpu * TPC:(cpu + 1) * TPC].T
        xt = np.ascontiguousarray(
            xs.reshape(NI, 128, TPC).transpose(1, 0, 2))
        in_maps.append({"xt": xt, "wt": wt, "wh": wh, "at": at, "a2t": a2t,
                        "b2t": b2t, "b2n": b2n, "mag": magr})
    return in_maps


def kernel(x, weight, lora_a_w, lora_b_w, magnitude, _trace=False, **_kw):
    nc = _get_program()
    in_maps = _prep_inputs(x, weight, lora_a_w, lora_b_w, magnitude)
    res = run_bass_kernel_spmd(nc, in_maps, list(range(NCORES)), trace=_trace)
    out = np.empty((TOK, DOUT), dtype=np.float32)
    for c in range(NCORES):
        out[c * TPC:(c + 1) * TPC] = res.results[c]["out"].T.astype(np.float32)
    if _trace:
        kernel._last_results = res
    return out.reshape(4, 2048, DOUT)
